# revision 2
# baseline (speedup 1.0000x reference)
"""Trainium2 Bass kernel for ConcatHandshaking.

out[b, p, :] = tanh(hidden[b, i_p] @ W1.T + hidden[b, j_p] @ W2.T + fc_b)
for the S*(S+1)/2 upper-triangular pairs (i_p, j_p), i-major order.

Device layout: output features (H=768) on SBUF partitions, pair index on the
free dim.  out[:, seg_i] = q2[:, i:256] + p1[:, i] (per-partition scalar), so
each triu segment is one DVE tensor_scalar_add, then one big ACT tanh per
output chunk, then one ~1MB DMA per chunk.

Throughput design (per core ~12.6M output elems):
- Everything after PSUM is bf16: DMA writes are halved (HBM ~358GB/s/core is
  the f32 bottleneck) and DVE adds run in 4x perf mode.
- 4x mode needs step-1, 4B-aligned, even-length APs on the bf16 operands, so
  segment lengths are padded to even (host discards pad cols) and odd-start
  reads use q2s, a one-column-shifted copy of q2.  The f32 per-partition
  scalar (p1[:, i]) is exempt from the 16-bit requirement.
- ACT (the 82us/core floor: 1 elem/cycle/lane @1.2GHz) runs only ~8 big-chunk
  tanh ops per stripe, avoiding per-segment instruction overhead (~224cyc/op).

Sharding (8 cores): core k handles batch b = k//2 and output-feature rows
[384*(k%2), 384*(k%2)+384) -> 3 stripes of [128 features, P pairs] each.
Per-core DRAM output is (3, 128, PPAD) bf16; host gathers the packed columns,
upcasts to f32 and transposes.
"""

import sys

import numpy as np

for _p in ("/opt/trn_rl_repo",):
    if _p not in sys.path:
        sys.path.insert(0, _p)

B, S, H = 4, 256, 768
P = S * (S + 1) // 2  # 32896
KT = H // 128  # 6 k-tiles
OC = 3  # o-chunks (of 128) per core
# bf16 packed matmul input columns: [ ht (S) | w1t (384) | w2t (384) ]
W1C = S
W2C = S + 128 * OC
IC16 = S + 2 * 128 * OC  # 1024

NCHUNK = 8  # tanh/DMA chunks per stripe (~1MB bf16 DMAs)

_NC_CACHE = {}
_LAYOUT_CACHE = {}
LAST = {}


def _layout(nchunk=NCHUNK, lead_split=True):
    """Padded segment layout + chunking.

    Segment i (i=0..S-1) holds pairs (i, j) for j=i..S-1, length L=S-i,
    padded to even Lp so every segment start (and the chunk starts) are
    4B-aligned in bf16.  Chunk boundaries snap to segment starts.

    Returns (PPAD, chunks, idx) where chunks is a list per stripe-chunk of
    (coff, csz, [(i, src_sel, src_off, Lp, dst0), ...]) and idx maps packed
    column -> padded column for the host-side gather.
    """
    key = (nchunk, lead_split)
    if key in _LAYOUT_CACHE:
        return _LAYOUT_CACHE[key]
    segs = []
    pos = 0
    for i in range(S):
        L = S - i
        Lp = L + (L & 1)
        segs.append((i, pos, L, Lp))
        pos += Lp
    ppad = pos  # 33024
    # chunk boundaries ~ppad/nchunk, snapped to segment starts
    targets = [round(ppad * t / nchunk) for t in range(1, nchunk)]
    bounds = [0]
    for t in targets:
        b = min((s[1] for s in segs), key=lambda x: abs(x - t))
        if b > bounds[-1]:
            bounds.append(b)
    bounds.append(ppad)
    if lead_split:
        # split the first chunk in half for earlier pipeline start
        half = min(
            (s[1] for s in segs if 0 < s[1] < bounds[1]),
            key=lambda x: abs(x - bounds[1] // 2),
        )
        bounds = [0, half] + bounds[1:]
    chunks = []
    idx = np.empty(P, dtype=np.int64)
    poff = 0
    si = 0
    for c0, c1 in zip(bounds[:-1], bounds[1:]):
        ops = []
        while si < len(segs) and segs[si][1] < c1:
            i, dstart, L, Lp = segs[si]
            idx[poff : poff + L] = dstart + np.arange(L)
            poff += L
            if i % 2 == 0:
                ops.append((i, 0, i, Lp, dstart - c0))
            else:
                ops.append((i, 1, i - 1, Lp, dstart - c0))
            si += 1
        chunks.append((c0, c1 - c0, ops))
    assert poff == P
    _LAYOUT_CACHE[key] = (ppad, chunks, idx)
    return _LAYOUT_CACHE[key]


def _build_nc(loop_k=None, nchunk=NCHUNK):
    import contextlib

    import concourse.bacc as bacc
    import concourse.bass as bass
    import concourse.mybir as mybir
    import concourse.tile as tile

    f32 = mybir.dt.float32
    bf16 = mybir.dt.bfloat16
    ppad, chunks, _ = _layout(nchunk)
    cmax = max(c[1] for c in chunks)

    # Bacc (not raw Bass): its compile() runs generate_event_semaphores,
    # which splits multi-sem waits to satisfy TRN2's 1-wait-per-instruction.
    nc = bacc.Bacc()

    inp16_d = nc.declare_dram_parameter("inp16", [H, IC16], bf16, isOutput=False)
    # f32 side data: col 0 = fcb (rows 0:384), col 1 = zeros
    aux_d = nc.declare_dram_parameter("aux", [H, 2], f32, isOutput=False)
    out_d = nc.declare_dram_parameter("out", [OC, 128, ppad], bf16, isOutput=True)

    Tanh = mybir.ActivationFunctionType.Tanh

    with tile.TileContext(nc) as tc:
        with (
            tc.tile_pool(name="const", bufs=1) as cpool,
            tc.tile_pool(name="mm", bufs=2, space="PSUM") as mpool,
            tc.tile_pool(name="sums", bufs=4) as spool,
            tc.tile_pool(name="outs", bufs=6) as opool,
            tc.For_i(0, loop_k, 1) if loop_k else contextlib.nullcontext(),
        ):
            # one DMA per k-tile so matmul kk can start as soon as its
            # k-tile lands (pipelines the load under the matmul chain)
            inp_b = cpool.tile([128, KT * IC16], bf16, name="inp_b")
            for kk in range(KT):
                nc.sync.dma_start(
                    inp_b[:, kk * IC16 : (kk + 1) * IC16],
                    inp16_d[kk * 128 : (kk + 1) * 128, :],
                )
            aux_b = cpool.tile([128, KT * 2], f32, name="aux_b")
            nc.sync.dma_start(
                aux_b[:].rearrange("p (t c) -> p t c", t=KT),
                aux_d.rearrange("(t p) c -> p t c", p=128),
            )
            # block kk occupies cols [kk*IC16, (kk+1)*IC16)
            ht_t = [inp_b[:, kk * IC16 : kk * IC16 + S] for kk in range(KT)]
            fcb_t = [aux_b[:, c * 2 : c * 2 + 1] for c in range(OC)]

            for c in range(OC):
                pm1 = mpool.tile([128, S], f32, name="pm1")
                pm2 = mpool.tile([128, S], f32, name="pm2")
                for kk in range(KT):
                    nc.tensor.matmul(
                        pm1[:],
                        inp_b[
                            :, kk * IC16 + W1C + c * 128 : kk * IC16 + W1C + (c + 1) * 128
                        ],
                        ht_t[kk],
                        start=(kk == 0),
                        stop=(kk == KT - 1),
                    )
                for kk in range(KT):
                    nc.tensor.matmul(
                        pm2[:],
                        inp_b[
                            :, kk * IC16 + W2C + c * 128 : kk * IC16 + W2C + (c + 1) * 128
                        ],
                        ht_t[kk],
                        start=(kk == 0),
                        stop=(kk == KT - 1),
                    )
                # p1 stays f32: the tensor_scalar per-partition scalar operand
                # is exempt from the 16-bit requirement of DVE 4x mode.
                p1 = cpool.tile([128, S], f32, name=f"p1_{c}")
                # q2 = bf16(pm2 + fcb); col S is a pad column read (only) by
                # the even-length extension of odd-L segments.
                q2 = cpool.tile([128, S + 1], bf16, name=f"q2_{c}")
                # q2s[k] = q2[k+1]: odd-i segments read q2s at even offset i-1
                q2s = cpool.tile([128, S], bf16, name=f"q2s_{c}")
                nc.vector.tensor_copy(p1[:], pm1[:])
                nc.vector.tensor_scalar_add(q2[:, :S], pm2[:], fcb_t[c])
                nc.vector.memset(q2[:, S : S + 1], 0.0)
                nc.vector.tensor_copy(q2s[:], q2[:, 1 : S + 1])

                for coff, csz, ops in chunks:
                    ot = spool.tile([128, cmax], bf16, name="ot")
                    ot2 = opool.tile([128, cmax], bf16, name="ot2")
                    for i, sel, soff, lp, d0 in ops:
                        src = q2s if sel else q2
                        nc.vector.tensor_scalar_add(
                            ot[:, d0 : d0 + lp],
                            src[:, soff : soff + lp],
                            p1[:, i : i + 1],
                        )
                    nc.scalar.activation(ot2[:, :csz], ot[:, :csz], Tanh)
                    nc.sync.dma_start(
                        out_d[c, :, coff : coff + csz], ot2[:, :csz]
                    )
    nc.compile()
    return nc


def _get_nc():
    if "nc" not in _NC_CACHE:
        _NC_CACHE["nc"] = _build_nc()
    return _NC_CACHE["nc"]


def _make_in_maps(hidden_state, fc_w, fc_b):
    import ml_dtypes

    in_maps = []
    for k in range(8):
        b, h0 = k // 2, 384 * (k % 2)
        inp16 = np.empty((H, IC16), dtype=ml_dtypes.bfloat16)
        inp16[:, :S] = hidden_state[b].T.astype(ml_dtypes.bfloat16)
        inp16[:, W1C : W1C + 384] = fc_w[h0 : h0 + 384, :H].T.astype(
            ml_dtypes.bfloat16
        )
        inp16[:, W2C : W2C + 384] = fc_w[h0 : h0 + 384, H:].T.astype(
            ml_dtypes.bfloat16
        )
        aux = np.zeros((H, 2), dtype=np.float32)
        aux[: 128 * OC, 0] = fc_b[h0 : h0 + 384]
        in_maps.append(dict(inp16=inp16, aux=aux))
    return in_maps


def kernel(hidden_state, fc_w, fc_b, _trace=False, **_trace_kwargs):
    from concourse.bass_utils import run_bass_kernel_spmd

    hidden_state = np.asarray(hidden_state, dtype=np.float32)
    fc_w = np.asarray(fc_w, dtype=np.float32)
    fc_b = np.asarray(fc_b, dtype=np.float32)

    in_maps = _make_in_maps(hidden_state, fc_w, fc_b)
    nc = _get_nc()
    res = run_bass_kernel_spmd(
        nc, in_maps, core_ids=list(range(8)), trace=_trace, **_trace_kwargs
    )
    LAST["res"] = res

    ppad, _, idx = _layout()
    full = np.empty((B, H, P), dtype=np.float32)
    for k in range(8):
        b, h0 = k // 2, 384 * (k % 2)
        o = np.asarray(res.results[k]["out"]).reshape(384, ppad)
        full[b, h0 : h0 + 384] = o[:, idx].astype(np.float32)
    return np.ascontiguousarray(full.transpose(0, 2, 1))
